# revision 55
# baseline (speedup 1.0000x reference)
"""Trainium2 Bass kernel for nn_ModalGenerator (MoE-routed cross-modal generator).

Strategy:
  - seq_len==1 => attention collapses to v = tgt @ wv.T; fold wv/ao_w into one
    512x512 matrix per layer (host-side) and (1-rw) into the output projection.
  - MoE routing on host: gather missing_type==1 columns (gen0) and ==2 (gen1);
    missing_type==3 rows use the tiny host-computed prior MLP.
  - Generator-split sharding: cores 0-3 run generator 0 on 1/4 of its columns
    each, cores 4-7 run generator 1. Halves per-core weight DMA vs
    data-parallel; zero collectives (host gathers/scatters).
  - All projections run in fp8-e4m3 DoubleRow mode (K=256 per pass, 2x PE
    rate): weights scaled x64 into e4m3's normal range, the 1/64 unscale
    folded into downstream ACT scales / residual scalars. src/tgt features
    are loaded as fp8 tile-major; the residual stream and LayerNorm math
    stay bf16 (rel err ~1.3e-2 vs the 2e-2 gate).
  - Layer 0 fuses the input projection into the attention PSUM group; layers
    1-2 fuse the residual add into the same PSUM group via a 64-scaled
    identity matmul, so PSUM->SBUF is a single ACT copy (scale=1/64).
  - LayerNorm: column sums via ones-matmul (broadcast across partitions),
    variance from q/H - m^2, rstd via bf16 bit-hack (logical_shift_right,
    int16 magic 0x5F37), all on DVE with 2x-packed bf16; the Newton
    refinement runs only on the last LN (which feeds the output projection)
    -- earlier LNs' per-column scale error renormalizes through later LNs;
    eps dropped (padded zero columns stay finite through the bit-hack).
  - Wavefront software pipelining: the program is a grid (stage, tile) with
    NT=4 staggered column tiles; stage s of tile t issues at wave s+t (later
    stages first within a wave), so each tile's serial LN chain hides behind
    other tiles' matmuls, including across layer boundaries.
  - PSUM: every matmul accumulation group owns full 2KB banks (start=True
    zeroes whole banks); 3 double-bank "mm" buffers + 1 "st" stats buffer
    = 8 banks. Engine balance: PE matmuls (incl. identity residuals); DVE
    ysq/LN chains/xf8; Pool (gpsimd) the off-critical-path xn of LN1; ACT
    gelu + PSUM->SBUF copies.
"""

import math

import numpy as np
import ml_dtypes

import concourse.bacc as bacc
import concourse.mybir as mybir
import concourse.tile as tile
from concourse.bass_utils import run_bass_kernel_spmd

f32 = mybir.dt.float32
bf16 = mybir.dt.bfloat16
f8 = mybir.dt.float8e4
i16 = mybir.dt.int16
AF = mybir.ActivationFunctionType
ALU = mybir.AluOpType
DR = mybir.MatmulPerfMode.DoubleRow

H = 512
L = 3
N_CORES = 8
GCORES = 4               # cores per generator
KC = H // 128            # 4 k-chunks of the hidden dim
FH = 4 * H               # 2048 FFN hidden
FKC = FH // 128          # 16
LN_EPS = 1e-5
MAGIC16 = 0x5F37
W8SCALE = 64.0           # fp8 weight pre-scale

FFN_FP8 = True
POW_RSTD = False         # rstd via ALU pow(-0.5); else bit-hack + Newton

# param pack column layout: [128, 128] f32
_P_IPB = 0               # unused when fused (ipb folded into ba[0])
_P_LAYER = 4             # + 40*l: ba 0..3 | f1b 4..19 | f2b 20..23
#                                 | ln1g 24..27 | ln1b 28..31 | ln2g 32..35 | ln2b 36..39
_P_OPB = 124


def _pack_pcol(vec):
    """[n*128] vector -> [128, n] chunk-column layout."""
    return np.ascontiguousarray(np.asarray(vec, np.float32).reshape(-1, 128).T)


def _sb_pack(wT, dt):
    """[K, M] (K mult of 128) -> [128, (K/128)*M] SBUF chunk-major layout."""
    K, M = wT.shape
    a = np.asarray(wT, np.float32).astype(dt)
    return np.ascontiguousarray(
        a.reshape(K // 128, 128, M).transpose(1, 0, 2).reshape(128, -1))


NT_TARGET = 4            # pipeline depth (equal column tiles per core)


def _tiles(C):
    nt = min(NT_TARGET, max(1, C // 64))
    base = C // nt // 16 * 16
    sizes = [base] * nt
    extra = C - base * nt
    i = 0
    while extra > 0:
        sizes[i] += min(16, extra)
        extra -= 16
        i = (i + 1) % nt
    # stagger: bigger tiles first (pipeline fill), smallest last (drain)
    if nt >= 2 and min(sizes) - 16 * (nt - 1) >= 64:
        sizes = [s + 16 * (nt - 1 - 2 * i) for i, s in enumerate(sizes)]
    sizes.sort(reverse=True)
    ts = []
    c0 = 0
    for s in sizes:
        if s > 0:
            ts.append((c0, c0 + s))
            c0 += s
    assert c0 == C
    return ts


def _build_program(C, skips):
    """skips = (zero_bias, unit_gamma, zero_beta) -- data-driven fast paths."""
    zb, ug, zbeta = skips
    f8dt = f8 if FFN_FP8 else bf16
    nc = bacc.Bacc("TRN2", target_bir_lowering=False, debug=False,
                   num_devices=N_CORES)

    dram = {
        "src": nc.dram_tensor("src", [128, KC * C], f8, kind="ExternalInput"),
        "tgt": nc.dram_tensor("tgt", [128, KC * C], f8, kind="ExternalInput"),
        "ip": nc.dram_tensor("ip", [128, KC * H], f8, kind="ExternalInput"),
        "op": nc.dram_tensor("op", [128, KC * H], f8, kind="ExternalInput"),
        "wa": nc.dram_tensor("wa", [L, 128, KC * H], f8, kind="ExternalInput"),
        "f1": nc.dram_tensor("f1", [L, 128, KC * FH], f8dt, kind="ExternalInput"),
        "f2": nc.dram_tensor("f2", [L, 128, FKC * H], f8dt, kind="ExternalInput"),
        "par": nc.dram_tensor("par", [128, 128], f32, kind="ExternalInput"),
        "ones": nc.dram_tensor("ones", [128, 128], bf16, kind="ExternalInput"),
        "ident": nc.dram_tensor("ident", [128, 128], bf16, kind="ExternalInput"),
        "out": nc.dram_tensor("out", [128, KC * C], bf16, kind="ExternalOutput"),
    }
    tiles = _tiles(C)
    NT = len(tiles)
    PB = min(NT, 3)
    unsc = 1.0 / W8SCALE if FFN_FP8 else 1.0

    with tile.TileContext(nc) as tc:
        with (
            tc.tile_pool(name="sb", bufs=2) as sb,
            tc.tile_pool(name="ps", bufs=2, space="PSUM") as psp,
        ):
            ipw = sb.tile([128, KC, H], f8, tag="ip", bufs=1)
            nc.sync.dma_start(ipw[:], dram["ip"].ap())
            wa0 = sb.tile([128, KC, H], f8, tag="wa", bufs=2)
            nc.sync.dma_start(wa0[:], dram["wa"].ap()[0])
            srcT = sb.tile([128, KC * C], f8, tag="src", bufs=1)
            tgtT = sb.tile([128, KC * C], f8, tag="tgt", bufs=1)
            nc.sync.dma_start(srcT[:, 0:KC * tiles[0][1]],
                              dram["src"].ap()[:, 0:KC * tiles[0][1]])
            nc.sync.dma_start(tgtT[:, 0:KC * tiles[0][1]],
                              dram["tgt"].ap()[:, 0:KC * tiles[0][1]])
            ones = sb.tile([128, 128], bf16, tag="ones", bufs=1)
            nc.sync.dma_start(ones[:], dram["ones"].ap())
            ident = sb.tile([128, 128], bf16, tag="ident", bufs=1)
            nc.sync.dma_start(ident[:], dram["ident"].ap())
            for ti in range(1, NT):
                c0, c1 = tiles[ti]
                nc.sync.dma_start(srcT[:, KC * c0:KC * c1],
                                  dram["src"].ap()[:, KC * c0:KC * c1])
                nc.sync.dma_start(tgtT[:, KC * c0:KC * c1],
                                  dram["tgt"].ap()[:, KC * c0:KC * c1])

            def _tm(flat, ti, p):
                c0, c1 = tiles[ti]
                Ct = c1 - c0
                sl = flat[:, KC * c0 + 2 * p * Ct:KC * c0 + (2 * p + 2) * Ct]
                return sl.rearrange("q (a b) -> q a b", a=2)
            par = sb.tile([128, 128], f32, tag="par", bufs=1)
            nc.sync.dma_start(par[:], dram["par"].ap())

            def stats_stage(y, Ct):
                """ACT square + PE column-sum matmuls -> st psum [s, q]."""
                ysq = sb.tile([128, KC, 512], bf16, tag="ysq", bufs=2)
                nc.vector.tensor_mul(ysq[:, :, 0:Ct], y[:, :, 0:Ct],
                                     y[:, :, 0:Ct])
                st = psp.tile([128, 2, 512], f32, tag="st", bufs=1)
                for k in range(KC):
                    nc.tensor.matmul(st[:, 0, 0:Ct], ones[:], y[:, k, 0:Ct],
                                     start=(k == 0), stop=(k == KC - 1))
                for k in range(KC):
                    nc.tensor.matmul(st[:, 1, 0:Ct], ones[:], ysq[:, k, 0:Ct],
                                     start=(k == 0), stop=(k == KC - 1))
                return st

            def ln_chain(y, st, Ct, gcol, bcol, xn, xf8, t, newton=True):
                """DVE scalar chain + apply: y,st -> xn bf16 [+ xf8 fp8]."""
                c0, c1 = t
                m = sb.tile([128, 512], bf16, tag="m", bufs=2)
                nc.vector.tensor_scalar(m[:, 0:Ct], st[:, 0, 0:Ct], 1.0 / H,
                                        None, ALU.mult)
                msq = sb.tile([128, 512], bf16, tag="msq", bufs=2)
                nc.vector.scalar_tensor_tensor(msq[:, 0:Ct], st[:, 0, 0:Ct],
                                               1.0 / H, m[:, 0:Ct],
                                               ALU.mult, ALU.mult)
                # z = q/H - m^2; eps dropped: padded all-zero columns stay
                # finite through the bit-hack (r^2 < bf16 max), real columns
                # have var >> eps.
                z = sb.tile([128, 512], bf16, tag="z", bufs=2)
                nc.vector.scalar_tensor_tensor(z[:, 0:Ct], st[:, 1, 0:Ct],
                                               1.0 / H, msq[:, 0:Ct],
                                               ALU.mult, ALU.subtract)
                r = sb.tile([128, 512], bf16, tag="rx", bufs=2)
                nc.vector.tensor_scalar(r[:, 0:Ct].bitcast(i16),
                                        z[:, 0:Ct].bitcast(i16), 1, None,
                                        ALU.logical_shift_right)
                rstd = sb.tile([128, 512], bf16, tag="rstd", bufs=4)
                nc.vector.tensor_scalar(rstd[:, 0:Ct].bitcast(i16),
                                        r[:, 0:Ct].bitcast(i16), -1,
                                        MAGIC16, ALU.mult, ALU.add)
                if newton:
                    u = sb.tile([128, 512], bf16, tag="u", bufs=2)
                    nc.vector.tensor_mul(u[:, 0:Ct], rstd[:, 0:Ct],
                                         rstd[:, 0:Ct])
                    w = sb.tile([128, 512], bf16, tag="w", bufs=2)
                    nc.vector.scalar_tensor_tensor(w[:, 0:Ct], u[:, 0:Ct],
                                                   -0.5, z[:, 0:Ct],
                                                   ALU.mult, ALU.mult)
                    rstd2 = sb.tile([128, 512], bf16, tag="rstd", bufs=4)
                    nc.vector.scalar_tensor_tensor(rstd2[:, 0:Ct], w[:, 0:Ct],
                                                   1.5, rstd[:, 0:Ct],
                                                   ALU.add, ALU.mult)
                else:
                    rstd2 = rstd
                m4 = m[:, 0:Ct].unsqueeze(1).broadcast_to((128, KC, Ct))
                r4 = rstd2[:, 0:Ct].unsqueeze(1).broadcast_to((128, KC, Ct))
                if ug and zbeta:
                    u1 = sb.tile([128, KC, 512], bf16, tag="u1", bufs=2)
                    nc.vector.tensor_sub(u1[:, :, 0:Ct], y[:, :, 0:Ct], m4)
                    if xf8 is not None:
                        nc.vector.tensor_mul(xf8[:, :, c0:c1],
                                             u1[:, :, 0:Ct], r4)
                        nc.gpsimd.tensor_mul(xn[:, :, c0:c1],
                                             u1[:, :, 0:Ct], r4)
                    else:
                        nc.vector.tensor_mul(xn[:, :, c0:c1],
                                             u1[:, :, 0:Ct], r4)
                else:
                    for mm in range(KC):
                        u1 = sb.tile([128, 512], bf16, tag="u1c", bufs=1)
                        nc.vector.tensor_sub(u1[:, 0:Ct], y[:, mm, 0:Ct],
                                             m[:, 0:Ct])
                        u2 = sb.tile([128, 512], bf16, tag="u2c", bufs=1)
                        nc.vector.scalar_tensor_tensor(
                            u2[:, 0:Ct], u1[:, 0:Ct],
                            par[:, gcol + mm:gcol + mm + 1], rstd2[:, 0:Ct],
                            ALU.mult, ALU.mult)
                        nc.vector.tensor_scalar(
                            xn[:, mm, c0:c1], u2[:, 0:Ct],
                            par[:, bcol + mm:bcol + mm + 1], None, ALU.add)
                    if xf8 is not None:
                        nc.scalar.activation(xf8[:, :, c0:c1], xn[:, :, c0:c1],
                                             AF.Copy)

            x = None
            xn_all, xf8_all = [], []
            for l in range(L):
                xn_a = sb.tile([128, KC, C], bf16, tag="x", bufs=3)
                xn_b = sb.tile([128, KC, C], bf16, tag="x", bufs=3)
                xn_all.append((xn_a, xn_b))
                if FFN_FP8:
                    xf8_t = sb.tile([128, KC, C], f8, tag="xf8", bufs=2)
                    xf8_all.append(xf8_t)
                else:
                    xf8_all.append(None)

            was, f1s, f2s = [wa0], [], []
            for l in range(L):
                if l > 0:
                    wa = sb.tile([128, KC, H], f8, tag="wa", bufs=2)
                    nc.sync.dma_start(wa[:], dram["wa"].ap()[l])
                    was.append(wa)
                f1w = sb.tile([128, KC, FH], f8dt, tag="f1", bufs=2)
                nc.sync.dma_start(f1w[:], dram["f1"].ap()[l])
                f1s.append(f1w)
                f2w = sb.tile([128, FKC, H], f8dt, tag="f2", bufs=2)
                nc.sync.dma_start(f2w[:], dram["f2"].ap()[l])
                f2s.append(f2w)
            opw = sb.tile([128, KC, H], f8, tag="op", bufs=1)
            nc.sync.dma_start(opw[:], dram["op"].ap())

            ys_all = [[None] * NT for _ in range(L)]
            y2s_all = [[None] * NT for _ in range(L)]
            hh_all = [[None] * NT for _ in range(L)]

            def attn_stage(l, ti):
                pb = _P_LAYER + 40 * l
                wa = was[l]
                xp = xn_all[l - 1][1] if l > 0 else None
                c0, c1 = tiles[ti]
                Ct = c1 - c0
                y = sb.tile([128, KC, 512], bf16, tag="y", bufs=NT)
                for g in range(2):
                    ps = psp.tile([128, 2, 512], f32, tag="mm", bufs=3)
                    for j in range(2):
                        mi = 2 * g + j
                        if l == 0:
                            for p in range(KC // 2):
                                nc.tensor.matmul(
                                    ps[:, j, 0:Ct],
                                    ipw[:, 2 * p:2 * p + 2,
                                        128 * mi:128 * (mi + 1)],
                                    _tm(srcT, ti, p),
                                    start=(p == 0), stop=False, perf_mode=DR)
                        for p in range(KC // 2):
                            nc.tensor.matmul(
                                ps[:, j, 0:Ct],
                                wa[:, 2 * p:2 * p + 2, 128 * mi:128 * (mi + 1)],
                                _tm(tgtT, ti, p),
                                start=(p == 0 and l != 0),
                                stop=(p == KC // 2 - 1 and l == 0),
                                perf_mode=DR)
                        if l != 0:
                            # residual via 64-scaled identity (unscale folds)
                            nc.tensor.matmul(
                                ps[:, j, 0:Ct], ident[:],
                                xp[:, mi, c0:c1],
                                start=False, stop=True)
                    if zb:
                        nc.scalar.activation(y[:, 2 * g:2 * g + 2, 0:Ct],
                                             ps[:, :, 0:Ct], AF.Copy,
                                             scale=unsc)
                    else:
                        for j in range(2):
                            mi = 2 * g + j
                            nc.scalar.activation(
                                y[:, mi, 0:Ct], ps[:, j, 0:Ct], AF.Copy,
                                scale=unsc,
                                bias=par[:, pb + mi:pb + mi + 1])
                ys_all[l][ti] = y

            def ln1_stage(l, ti):
                pb = _P_LAYER + 40 * l
                t = tiles[ti]
                st = stats_stage(ys_all[l][ti], t[1] - t[0])
                ln_chain(ys_all[l][ti], st, t[1] - t[0], pb + 24, pb + 28,
                         xn_all[l][0], xf8_all[l], t, newton=False)

            def f1_stage(l, ti):
                pb = _P_LAYER + 40 * l
                f1w = f1s[l]
                xin = xf8_all[l] if FFN_FP8 else xn_all[l][0]
                c0, c1 = tiles[ti]
                Ct = c1 - c0
                hh = sb.tile([128, FKC, 512], f8dt, tag="h", bufs=2)
                for g in range(FKC // 2):
                    ps = psp.tile([128, 2, 512], f32, tag="mm", bufs=3)
                    for j in range(2):
                        mi = 2 * g + j
                        if FFN_FP8:
                            for p in range(KC // 2):
                                nc.tensor.matmul(
                                    ps[:, j, 0:Ct],
                                    f1w[:, 2 * p:2 * p + 2,
                                        128 * mi:128 * (mi + 1)],
                                    xin[:, 2 * p:2 * p + 2, c0:c1],
                                    start=(p == 0), stop=(p == KC // 2 - 1),
                                    perf_mode=DR)
                        else:
                            for k in range(KC):
                                nc.tensor.matmul(
                                    ps[:, j, 0:Ct],
                                    f1w[:, k, 128 * mi:128 * (mi + 1)],
                                    xin[:, k, c0:c1],
                                    start=(k == 0), stop=(k == KC - 1))
                    if zb:
                        nc.scalar.activation(hh[:, 2 * g:2 * g + 2, 0:Ct],
                                             ps[:, :, 0:Ct], AF.Gelu,
                                             scale=unsc)
                    else:
                        for j in range(2):
                            mi = 2 * g + j
                            nc.scalar.activation(
                                hh[:, mi, 0:Ct], ps[:, j, 0:Ct], AF.Gelu,
                                scale=unsc,
                                bias=par[:, pb + 4 + mi:pb + 4 + mi + 1])
                hh_all[l][ti] = hh

            def f2_stage(l, ti):
                pb = _P_LAYER + 40 * l
                f2w = f2s[l]
                xn = xn_all[l][0]
                hh = hh_all[l][ti]
                c0, c1 = tiles[ti]
                Ct = c1 - c0
                y2 = sb.tile([128, KC, 512], bf16, tag="y", bufs=NT)
                for g in range(2):
                    ps = psp.tile([128, 2, 512], f32, tag="mm", bufs=3)
                    for j in range(2):
                        mi = 2 * g + j
                        if FFN_FP8:
                            for p in range(FKC // 2):
                                nc.tensor.matmul(
                                    ps[:, j, 0:Ct],
                                    f2w[:, 2 * p:2 * p + 2,
                                        128 * mi:128 * (mi + 1)],
                                    hh[:, 2 * p:2 * p + 2, 0:Ct],
                                    start=(p == 0), stop=False,
                                    perf_mode=DR)
                        else:
                            for k in range(FKC):
                                nc.tensor.matmul(
                                    ps[:, j, 0:Ct],
                                    f2w[:, k, 128 * mi:128 * (mi + 1)],
                                    hh[:, k, 0:Ct],
                                    start=(k == 0), stop=False)
                        nc.tensor.matmul(
                            ps[:, j, 0:Ct], ident[:], xn[:, mi, c0:c1],
                            start=False, stop=True)
                    if zb:
                        nc.scalar.activation(y2[:, 2 * g:2 * g + 2, 0:Ct],
                                             ps[:, :, 0:Ct], AF.Copy,
                                             scale=unsc)
                    else:
                        for j in range(2):
                            mi = 2 * g + j
                            nc.scalar.activation(
                                y2[:, mi, 0:Ct], ps[:, j, 0:Ct], AF.Copy,
                                scale=unsc,
                                bias=par[:, pb + 20 + mi:pb + 20 + mi + 1])
                y2s_all[l][ti] = y2

            def ln2_stage(l, ti):
                pb = _P_LAYER + 40 * l
                t = tiles[ti]
                st2 = stats_stage(y2s_all[l][ti], t[1] - t[0])
                ln_chain(y2s_all[l][ti], st2, t[1] - t[0], pb + 32, pb + 36,
                         xn_all[l][1],
                         xf8_all[l] if l == L - 1 else None, t,
                         newton=(l == L - 1))

            def out_stage(l, ti):
                xo = xf8_all[L - 1]
                c0, c1 = tiles[ti]
                Ct = c1 - c0
                for g in range(2):
                    ps = psp.tile([128, 2, 512], f32, tag="mm", bufs=3)
                    for j in range(2):
                        mi = 2 * g + j
                        for p in range(KC // 2):
                            nc.tensor.matmul(
                                ps[:, j, 0:Ct],
                                opw[:, 2 * p:2 * p + 2,
                                    128 * mi:128 * (mi + 1)],
                                xo[:, 2 * p:2 * p + 2, c0:c1],
                                start=(p == 0), stop=(p == KC // 2 - 1),
                                perf_mode=DR)
                    ot = sb.tile([128, 2, 512], bf16, tag="o", bufs=2)
                    if zb:
                        nc.scalar.activation(ot[:, :, 0:Ct], ps[:, :, 0:Ct],
                                             AF.Copy, scale=unsc)
                    else:
                        for j in range(2):
                            mi = 2 * g + j
                            nc.scalar.activation(
                                ot[:, j, 0:Ct], ps[:, j, 0:Ct], AF.Copy,
                                scale=unsc,
                                bias=par[:, _P_OPB + mi:_P_OPB + mi + 1])
                    for j in range(2):
                        mi = 2 * g + j
                        nc.sync.dma_start(
                            dram["out"].ap()[:, mi * C + c0:mi * C + c1],
                            ot[:, j, 0:Ct])

            # wavefront issue: stage ls of tile t at wave ls + t
            stage_fns = []
            for l in range(L):
                stage_fns += [
                    (attn_stage, l), (ln1_stage, l), (f1_stage, l),
                    (f2_stage, l), (ln2_stage, l),
                ]
            stage_fns.append((out_stage, L - 1))
            NS = len(stage_fns)
            for wave in range(NS + NT - 1):
                for ls in range(NS - 1, -1, -1):
                    ti = wave - ls
                    if 0 <= ti < NT:
                        fn, l = stage_fns[ls]
                        fn(l, ti)

    nc.compile()
    return nc


_CACHE = {}


def _get_program(C, skips):
    key = (C, skips)
    if key not in _CACHE:
        _CACHE[key] = _build_program(C, skips)
    return _CACHE[key]


def _prep_gen_weights(i, g_ipw, g_ipb, g_qkv_w, g_qkv_b, g_ao_w, g_ao_b,
                      g_ln1g, g_ln1b, g_ln2g, g_ln2b, g_f1w, g_f1b, g_f2w,
                      g_f2b, g_opw, g_opb, g_rw):
    wa, ba = [], []
    for l in range(L):
        _wq, _wk, wv = np.split(g_qkv_w[i, l], 3, axis=0)
        _bq, _bk, bv = np.split(g_qkv_b[i, l], 3)
        wa.append((g_ao_w[i, l] @ wv).T)                 # [K=H, M=H]
        ba.append(g_ao_b[i, l] + bv @ g_ao_w[i, l].T)
    rw = float(g_rw[i])
    ws = W8SCALE if FFN_FP8 else 1.0
    f8np = mybir.dt.np(f8 if FFN_FP8 else bf16)
    ipP = _sb_pack(W8SCALE * g_ipw[i].T, mybir.dt.np(f8))
    opP = _sb_pack(W8SCALE * (1.0 - rw) * g_opw[i].T, mybir.dt.np(f8))
    waP = np.stack([_sb_pack(W8SCALE * wa[l], mybir.dt.np(f8))
                    for l in range(L)])
    f1P = np.stack([_sb_pack(ws * g_f1w[i, l].T, f8np) for l in range(L)])
    f2P = np.stack([_sb_pack(ws * g_f2w[i, l].T, f8np) for l in range(L)])

    par = np.zeros((128, 128), np.float32)
    par[:, _P_IPB:_P_IPB + KC] = _pack_pcol(g_ipb[i])
    for l in range(L):
        pb = _P_LAYER + 40 * l
        bal = ba[l] + (g_ipb[i] if l == 0 else 0.0)   # layer-0 fuses ipb
        par[:, pb:pb + 4] = _pack_pcol(bal)
        par[:, pb + 4:pb + 20] = _pack_pcol(g_f1b[i, l])
        par[:, pb + 20:pb + 24] = _pack_pcol(g_f2b[i, l])
        par[:, pb + 24:pb + 28] = _pack_pcol(g_ln1g[i, l])
        par[:, pb + 28:pb + 32] = _pack_pcol(g_ln1b[i, l])
        par[:, pb + 32:pb + 36] = _pack_pcol(g_ln2g[i, l])
        par[:, pb + 36:pb + 40] = _pack_pcol(g_ln2b[i, l])
    par[:, _P_OPB:_P_OPB + KC] = _pack_pcol((1.0 - rw) * g_opb[i])

    zb = bool(np.all(g_ipb[i] == 0) and all(np.all(b == 0) for b in ba)
              and np.all(g_f1b[i] == 0) and np.all(g_f2b[i] == 0)
              and np.all(g_opb[i] == 0))
    ug = bool(np.all(g_ln1g[i] == 1) and np.all(g_ln2g[i] == 1))
    zbeta = bool(np.all(g_ln1b[i] == 0) and np.all(g_ln2b[i] == 0))
    return {"ip": ipP, "op": opP, "wa": waP, "f1": f1P, "f2": f2P,
            "par": par}, (zb, ug, zbeta), rw


def _prepare(inputs):
    """Host-side prep. Returns (nc, in_maps, assemble)."""
    image = np.asarray(inputs["image_features"], np.float32)
    text = np.asarray(inputs["text_features"], np.float32)
    mt = np.asarray(inputs["missing_type"])

    idx1 = np.nonzero(mt == 1)[0]      # gen0 (img -> text) fills text
    idx2 = np.nonzero(mt == 2)[0]      # gen1 (text -> img) fills img
    idx3 = np.nonzero(mt == 3)[0]

    gw = {k: np.asarray(v) for k, v in inputs.items() if k.startswith("g_")}
    w0, skips0, rw0 = _prep_gen_weights(0, **gw)
    w1, skips1, rw1 = _prep_gen_weights(1, **gw)
    skips = tuple(a and b for a, b in zip(skips0, skips1))

    # prior MLP on host (tiny)
    pe = np.asarray(inputs["prior_emb"], np.float64)
    t = pe @ np.asarray(inputs["prior_w1"], np.float64).T \
        + np.asarray(inputs["prior_b1"], np.float64)
    t = 0.5 * t * (1.0 + np.vectorize(math.erf)(t / math.sqrt(2.0)))
    prior = (t @ np.asarray(inputs["prior_w2"], np.float64).T
             + np.asarray(inputs["prior_b2"], np.float64)).astype(np.float32)
    p_img, p_text = prior[0, :H], prior[0, H:]

    imgT = np.ascontiguousarray(image.T)
    textT = np.ascontiguousarray(text.T)

    n_pc = -(-max(len(idx1), len(idx2), 1) // GCORES)   # per-core columns
    C = max(64, -(-n_pc // 16) * 16)                    # round up to 16

    tls = _tiles(C)

    def _pack_tm(M):
        """[H, C] -> tile-major [128, NT*KC*Tt] fp8."""
        a = M.astype(mybir.dt.np(f8)).reshape(KC, 128, C).transpose(1, 0, 2)
        return np.concatenate(
            [np.ascontiguousarray(a[:, :, t0:t1]).reshape(128, -1)
             for t0, t1 in tls], axis=1)

    def shard_cols(Tsrc, Ttgt, idx):
        pad = np.zeros(GCORES * C, np.int64)
        pad[:len(idx)] = idx
        pad = pad.reshape(GCORES, C)
        return [_pack_tm(Tsrc[:, pad[c]]) for c in range(GCORES)], \
            [_pack_tm(Ttgt[:, pad[c]]) for c in range(GCORES)]

    src0, tgt0 = shard_cols(imgT, textT, idx1)
    src1, tgt1 = shard_cols(textT, imgT, idx2)

    nc = _get_program(C, skips)

    ones = np.ones((128, 128), ml_dtypes.bfloat16)
    ident = (np.eye(128, dtype=np.float32) * W8SCALE).astype(ml_dtypes.bfloat16)
    in_maps = []
    for c in range(N_CORES):
        g = 0 if c < GCORES else 1
        w = w0 if g == 0 else w1
        lc = c % GCORES
        in_maps.append({
            "src": (src0 if g == 0 else src1)[lc],
            "tgt": (tgt0 if g == 0 else tgt1)[lc],
            "ip": w["ip"], "op": w["op"], "wa": w["wa"], "f1": w["f1"],
            "f2": w["f2"],
            "par": w["par"], "ones": ones, "ident": ident,
        })

    def assemble(results):
        def gather_out(cores, idx, rw, full):
            cols = [np.asarray(results[c]["out"])
                    .astype(np.float32)
                    .reshape(128, KC, C).transpose(1, 0, 2).reshape(H, C)
                    for c in cores]
            allc = np.concatenate(cols, axis=1)[:, :len(idx)]
            return rw * full[idx] + allc.T

        enhanced_text = text.copy()
        if len(idx1):
            enhanced_text[idx1] = gather_out(range(GCORES), idx1, rw0, text)
        enhanced_img = image.copy()
        if len(idx2):
            enhanced_img[idx2] = gather_out(range(GCORES, N_CORES), idx2,
                                            rw1, image)
        if len(idx3):
            enhanced_img[idx3] = p_img
            enhanced_text[idx3] = p_text
        return enhanced_img, enhanced_text

    return nc, in_maps, assemble


def kernel(**inputs):
    nc, in_maps, assemble = _prepare(inputs)
    res = run_bass_kernel_spmd(nc, in_maps, list(range(N_CORES)))
    return assemble(res.results)


# revision 57
# speedup vs baseline: 1.0201x; 1.0201x over previous
"""Trainium2 Bass kernel for nn_ModalGenerator (MoE-routed cross-modal generator).

Strategy:
  - seq_len==1 => attention collapses to v = tgt @ wv.T; fold wv/ao_w into one
    512x512 matrix per layer (host-side) and (1-rw) into the output projection.
  - MoE routing on host: gather missing_type==1 columns (gen0) and ==2 (gen1);
    missing_type==3 rows use the tiny host-computed prior MLP.
  - Generator-split sharding: cores 0-3 run generator 0 on 1/4 of its columns
    each, cores 4-7 run generator 1. Halves per-core weight DMA vs
    data-parallel; zero collectives (host gathers/scatters).
  - All projections run in fp8-e4m3 DoubleRow mode (K=256 per pass, 2x PE
    rate): weights scaled x64 into e4m3's normal range, the 1/64 unscale
    folded into downstream ACT scales / residual scalars. src/tgt features
    are loaded as fp8 tile-major; the residual stream and LayerNorm math
    stay bf16 (rel err ~1.3e-2 vs the 2e-2 gate).
  - Layer 0 fuses the input projection into the attention PSUM group; layers
    1-2 fuse the residual add into the same PSUM group via a 64-scaled
    identity matmul, so PSUM->SBUF is a single ACT copy (scale=1/64).
  - LayerNorm: column sums via ones-matmul (broadcast across partitions),
    variance from q/H - m^2, rstd via bf16 bit-hack (logical_shift_right,
    int16 magic 0x5F37), all on DVE with 2x-packed bf16; the Newton
    refinement runs only on the last LN (which feeds the output projection)
    -- earlier LNs' per-column scale error renormalizes through later LNs;
    eps dropped (padded zero columns stay finite through the bit-hack).
  - Wavefront software pipelining: the program is a grid (stage, tile) with
    NT=4 staggered column tiles; stage s of tile t issues at wave s+t (later
    stages first within a wave), so each tile's serial LN chain hides behind
    other tiles' matmuls, including across layer boundaries.
  - PSUM: every matmul accumulation group owns full 2KB banks (start=True
    zeroes whole banks); 3 double-bank "mm" buffers + 1 "st" stats buffer
    = 8 banks. Engine balance: PE matmuls (incl. identity residuals); DVE
    ysq/LN chains/xf8; Pool (gpsimd) the off-critical-path xn of LN1; ACT
    gelu + PSUM->SBUF copies.
"""

import math

import numpy as np
import ml_dtypes

import concourse.bacc as bacc
import concourse.mybir as mybir
import concourse.tile as tile
from concourse.bass_utils import run_bass_kernel_spmd

f32 = mybir.dt.float32
bf16 = mybir.dt.bfloat16
f8 = mybir.dt.float8e4
i16 = mybir.dt.int16
AF = mybir.ActivationFunctionType
ALU = mybir.AluOpType
DR = mybir.MatmulPerfMode.DoubleRow

H = 512
L = 3
N_CORES = 8
GCORES = 4               # cores per generator
KC = H // 128            # 4 k-chunks of the hidden dim
FH = 4 * H               # 2048 FFN hidden
FKC = FH // 128          # 16
LN_EPS = 1e-5
MAGIC16 = 0x5F37
W8SCALE = 64.0           # fp8 weight pre-scale

FFN_FP8 = True
POW_RSTD = False         # rstd via ALU pow(-0.5); else bit-hack + Newton

# param pack column layout: [128, 128] f32
_P_IPB = 0               # unused when fused (ipb folded into ba[0])
_P_LAYER = 4             # + 40*l: ba 0..3 | f1b 4..19 | f2b 20..23
#                                 | ln1g 24..27 | ln1b 28..31 | ln2g 32..35 | ln2b 36..39
_P_OPB = 124


def _pack_pcol(vec):
    """[n*128] vector -> [128, n] chunk-column layout."""
    return np.ascontiguousarray(np.asarray(vec, np.float32).reshape(-1, 128).T)


def _sb_pack(wT, dt):
    """[K, M] (K mult of 128) -> [128, (K/128)*M] SBUF chunk-major layout."""
    K, M = wT.shape
    a = np.asarray(wT, np.float32).astype(dt)
    return np.ascontiguousarray(
        a.reshape(K // 128, 128, M).transpose(1, 0, 2).reshape(128, -1))


NT_TARGET = 5            # pipeline depth (equal column tiles per core)


def _tiles(C):
    nt = min(NT_TARGET, max(1, C // 64))
    base = C // nt // 16 * 16
    sizes = [base] * nt
    extra = C - base * nt
    i = 0
    while extra > 0:
        sizes[i] += min(16, extra)
        extra -= 16
        i = (i + 1) % nt
    # stagger: bigger first tile (fill), smaller last (drain); cap 256 so
    # 4 psum chunks pack into 2 banks
    sizes.sort(reverse=True)
    if nt >= 2 and sizes[-1] >= 80 and sizes[0] <= 240:
        sizes[0] += 16
        sizes[-1] -= 16
    assert sizes[0] <= 256
    ts = []
    c0 = 0
    for s in sizes:
        if s > 0:
            ts.append((c0, c0 + s))
            c0 += s
    assert c0 == C
    return ts


def _build_program(C, skips):
    """skips = (zero_bias, unit_gamma, zero_beta) -- data-driven fast paths."""
    zb, ug, zbeta = skips
    f8dt = f8 if FFN_FP8 else bf16
    nc = bacc.Bacc("TRN2", target_bir_lowering=False, debug=False,
                   num_devices=N_CORES)

    dram = {
        "src": nc.dram_tensor("src", [128, KC * C], f8, kind="ExternalInput"),
        "tgt": nc.dram_tensor("tgt", [128, KC * C], f8, kind="ExternalInput"),
        "ip": nc.dram_tensor("ip", [128, KC * H], f8, kind="ExternalInput"),
        "op": nc.dram_tensor("op", [128, KC * H], f8, kind="ExternalInput"),
        "wa": nc.dram_tensor("wa", [L, 128, KC * H], f8, kind="ExternalInput"),
        "f1": nc.dram_tensor("f1", [L, 128, KC * FH], f8dt, kind="ExternalInput"),
        "f2": nc.dram_tensor("f2", [L, 128, FKC * H], f8dt, kind="ExternalInput"),
        "par": nc.dram_tensor("par", [128, 128], f32, kind="ExternalInput"),
        "ones": nc.dram_tensor("ones", [128, 128], bf16, kind="ExternalInput"),
        "ident": nc.dram_tensor("ident", [128, 128], bf16, kind="ExternalInput"),
        "out": nc.dram_tensor("out", [128, KC * C], bf16, kind="ExternalOutput"),
    }
    tiles = _tiles(C)
    NT = len(tiles)
    PB = min(NT, 3)
    unsc = 1.0 / W8SCALE if FFN_FP8 else 1.0

    with tile.TileContext(nc) as tc:
        with (
            tc.tile_pool(name="sb", bufs=2) as sb,
            tc.tile_pool(name="ps", bufs=2, space="PSUM") as psp,
        ):
            ipw = sb.tile([128, KC, H], f8, tag="ip", bufs=1)
            nc.sync.dma_start(ipw[:], dram["ip"].ap())
            wa0 = sb.tile([128, KC, H], f8, tag="wa", bufs=2)
            nc.sync.dma_start(wa0[:], dram["wa"].ap()[0])
            srcT = sb.tile([128, KC * C], f8, tag="src", bufs=1)
            tgtT = sb.tile([128, KC * C], f8, tag="tgt", bufs=1)
            nc.sync.dma_start(srcT[:, 0:KC * tiles[0][1]],
                              dram["src"].ap()[:, 0:KC * tiles[0][1]])
            nc.sync.dma_start(tgtT[:, 0:KC * tiles[0][1]],
                              dram["tgt"].ap()[:, 0:KC * tiles[0][1]])
            ones = sb.tile([128, 128], bf16, tag="ones", bufs=1)
            nc.sync.dma_start(ones[:], dram["ones"].ap())
            ident = sb.tile([128, 128], bf16, tag="ident", bufs=1)
            nc.sync.dma_start(ident[:], dram["ident"].ap())
            for ti in range(1, NT):
                c0, c1 = tiles[ti]
                nc.sync.dma_start(srcT[:, KC * c0:KC * c1],
                                  dram["src"].ap()[:, KC * c0:KC * c1])
                nc.sync.dma_start(tgtT[:, KC * c0:KC * c1],
                                  dram["tgt"].ap()[:, KC * c0:KC * c1])

            def _tm(flat, ti, p):
                c0, c1 = tiles[ti]
                Ct = c1 - c0
                sl = flat[:, KC * c0 + 2 * p * Ct:KC * c0 + (2 * p + 2) * Ct]
                return sl.rearrange("q (a b) -> q a b", a=2)
            par = sb.tile([128, 128], f32, tag="par", bufs=1)
            nc.sync.dma_start(par[:], dram["par"].ap())

            def stats_stage(y, Ct):
                """ACT square + PE column-sum matmuls -> st psum [s, q]."""
                ysq = sb.tile([128, KC, 512], bf16, tag="ysq", bufs=2)
                nc.vector.tensor_mul(ysq[:, :, 0:Ct], y[:, :, 0:Ct],
                                     y[:, :, 0:Ct])
                st = psp.tile([128, 2, 256], f32, tag="st", bufs=2)
                for k in range(KC):
                    nc.tensor.matmul(st[:, 0, 0:Ct], ones[:], y[:, k, 0:Ct],
                                     start=(k == 0), stop=False,
                                     skip_group_check=True)
                for k in range(KC):
                    nc.tensor.matmul(st[:, 1, 0:Ct], ones[:], ysq[:, k, 0:Ct],
                                     start=False, stop=(k == KC - 1),
                                     skip_group_check=True)
                return st

            def ln_chain(y, st, Ct, gcol, bcol, xn, xf8, t, newton=True):
                """DVE scalar chain + apply: y,st -> xn bf16 [+ xf8 fp8]."""
                c0, c1 = t
                m = sb.tile([128, 512], bf16, tag="m", bufs=2)
                nc.vector.tensor_scalar(m[:, 0:Ct], st[:, 0, 0:Ct], 1.0 / H,
                                        None, ALU.mult)
                msq = sb.tile([128, 512], bf16, tag="msq", bufs=2)
                nc.vector.scalar_tensor_tensor(msq[:, 0:Ct], st[:, 0, 0:Ct],
                                               1.0 / H, m[:, 0:Ct],
                                               ALU.mult, ALU.mult)
                # z = q/H - m^2; eps dropped: padded all-zero columns stay
                # finite through the bit-hack (r^2 < bf16 max), real columns
                # have var >> eps.
                z = sb.tile([128, 512], bf16, tag="z", bufs=2)
                nc.vector.scalar_tensor_tensor(z[:, 0:Ct], st[:, 1, 0:Ct],
                                               1.0 / H, msq[:, 0:Ct],
                                               ALU.mult, ALU.subtract)
                r = sb.tile([128, 512], bf16, tag="rx", bufs=2)
                nc.vector.tensor_scalar(r[:, 0:Ct].bitcast(i16),
                                        z[:, 0:Ct].bitcast(i16), 1, None,
                                        ALU.logical_shift_right)
                rstd = sb.tile([128, 512], bf16, tag="rstd", bufs=4)
                nc.vector.tensor_scalar(rstd[:, 0:Ct].bitcast(i16),
                                        r[:, 0:Ct].bitcast(i16), -1,
                                        MAGIC16, ALU.mult, ALU.add)
                if newton:
                    u = sb.tile([128, 512], bf16, tag="u", bufs=2)
                    nc.vector.tensor_mul(u[:, 0:Ct], rstd[:, 0:Ct],
                                         rstd[:, 0:Ct])
                    w = sb.tile([128, 512], bf16, tag="w", bufs=2)
                    nc.vector.scalar_tensor_tensor(w[:, 0:Ct], u[:, 0:Ct],
                                                   -0.5, z[:, 0:Ct],
                                                   ALU.mult, ALU.mult)
                    rstd2 = sb.tile([128, 512], bf16, tag="rstd", bufs=4)
                    nc.vector.scalar_tensor_tensor(rstd2[:, 0:Ct], w[:, 0:Ct],
                                                   1.5, rstd[:, 0:Ct],
                                                   ALU.add, ALU.mult)
                else:
                    rstd2 = rstd
                m4 = m[:, 0:Ct].unsqueeze(1).broadcast_to((128, KC, Ct))
                r4 = rstd2[:, 0:Ct].unsqueeze(1).broadcast_to((128, KC, Ct))
                if ug and zbeta:
                    u1 = sb.tile([128, KC, 512], bf16, tag="u1", bufs=2)
                    nc.vector.tensor_sub(u1[:, :, 0:Ct], y[:, :, 0:Ct], m4)
                    if xf8 is not None:
                        nc.vector.tensor_mul(xf8[:, :, c0:c1],
                                             u1[:, :, 0:Ct], r4)
                        nc.gpsimd.tensor_mul(xn[:, :, c0:c1],
                                             u1[:, :, 0:Ct], r4)
                    else:
                        nc.vector.tensor_mul(xn[:, :, c0:c1],
                                             u1[:, :, 0:Ct], r4)
                else:
                    for mm in range(KC):
                        u1 = sb.tile([128, 512], bf16, tag="u1c", bufs=1)
                        nc.vector.tensor_sub(u1[:, 0:Ct], y[:, mm, 0:Ct],
                                             m[:, 0:Ct])
                        u2 = sb.tile([128, 512], bf16, tag="u2c", bufs=1)
                        nc.vector.scalar_tensor_tensor(
                            u2[:, 0:Ct], u1[:, 0:Ct],
                            par[:, gcol + mm:gcol + mm + 1], rstd2[:, 0:Ct],
                            ALU.mult, ALU.mult)
                        nc.vector.tensor_scalar(
                            xn[:, mm, c0:c1], u2[:, 0:Ct],
                            par[:, bcol + mm:bcol + mm + 1], None, ALU.add)
                    if xf8 is not None:
                        nc.scalar.activation(xf8[:, :, c0:c1], xn[:, :, c0:c1],
                                             AF.Copy)

            x = None
            xn_all, xf8_all = [], []
            for l in range(L):
                xn_a = sb.tile([128, KC, C], bf16, tag="x", bufs=3)
                xn_b = sb.tile([128, KC, C], bf16, tag="x", bufs=3)
                xn_all.append((xn_a, xn_b))
                if FFN_FP8:
                    xf8_t = sb.tile([128, KC, C], f8, tag="xf8", bufs=2)
                    xf8_all.append(xf8_t)
                else:
                    xf8_all.append(None)

            was, f1s, f2s = [wa0], [], []
            for l in range(L):
                if l > 0:
                    wa = sb.tile([128, KC, H], f8, tag="wa", bufs=2)
                    nc.sync.dma_start(wa[:], dram["wa"].ap()[l])
                    was.append(wa)
                f1w = sb.tile([128, KC, FH], f8dt, tag="f1", bufs=2)
                nc.sync.dma_start(f1w[:], dram["f1"].ap()[l])
                f1s.append(f1w)
                f2w = sb.tile([128, FKC, H], f8dt, tag="f2", bufs=2)
                nc.sync.dma_start(f2w[:], dram["f2"].ap()[l])
                f2s.append(f2w)
            opw = sb.tile([128, KC, H], f8, tag="op", bufs=1)
            nc.sync.dma_start(opw[:], dram["op"].ap())

            ys_all = [[None] * NT for _ in range(L)]
            y2s_all = [[None] * NT for _ in range(L)]
            hh_all = [[None] * NT for _ in range(L)]

            def attn_stage(l, ti):
                pb = _P_LAYER + 40 * l
                wa = was[l]
                xp = xn_all[l - 1][1] if l > 0 else None
                c0, c1 = tiles[ti]
                Ct = c1 - c0
                y = sb.tile([128, KC, 512], bf16, tag="y", bufs=NT)
                # 4 chunks packed 2-per-bank: only the first matmul touching
                # each bank carries start=True (whole-bank zero covers the
                # sibling chunk, which accumulates onto zeros)
                ps = psp.tile([128, 4, 256], f32, tag="mm", bufs=3)
                for j in range(KC):
                    first = (j % 2 == 0)
                    last = (j % 2 == 1)
                    if l == 0:
                        for p in range(KC // 2):
                            nc.tensor.matmul(
                                ps[:, j, 0:Ct],
                                ipw[:, 2 * p:2 * p + 2, 128 * j:128 * (j + 1)],
                                _tm(srcT, ti, p),
                                start=(p == 0 and first), stop=False,
                                perf_mode=DR, skip_group_check=True)
                        for p in range(KC // 2):
                            nc.tensor.matmul(
                                ps[:, j, 0:Ct],
                                wa[:, 2 * p:2 * p + 2, 128 * j:128 * (j + 1)],
                                _tm(tgtT, ti, p),
                                start=False,
                                stop=(p == KC // 2 - 1 and last),
                                perf_mode=DR, skip_group_check=True)
                    else:
                        for p in range(KC // 2):
                            nc.tensor.matmul(
                                ps[:, j, 0:Ct],
                                wa[:, 2 * p:2 * p + 2, 128 * j:128 * (j + 1)],
                                _tm(tgtT, ti, p),
                                start=(p == 0 and first), stop=False,
                                perf_mode=DR, skip_group_check=True)
                        # residual via 64-scaled identity (unscale folds)
                        nc.tensor.matmul(
                            ps[:, j, 0:Ct], ident[:], xp[:, j, c0:c1],
                            start=False, stop=last, skip_group_check=True)
                if zb:
                    nc.scalar.activation(y[:, :, 0:Ct], ps[:, :, 0:Ct],
                                         AF.Copy, scale=unsc)
                else:
                    for j in range(KC):
                        nc.scalar.activation(
                            y[:, j, 0:Ct], ps[:, j, 0:Ct], AF.Copy,
                            scale=unsc, bias=par[:, pb + j:pb + j + 1])
                ys_all[l][ti] = y

            def ln1_stage(l, ti):
                pb = _P_LAYER + 40 * l
                t = tiles[ti]
                st = stats_stage(ys_all[l][ti], t[1] - t[0])
                ln_chain(ys_all[l][ti], st, t[1] - t[0], pb + 24, pb + 28,
                         xn_all[l][0], xf8_all[l], t, newton=False)

            def _f1_half(l, ti, half):
                pb = _P_LAYER + 40 * l
                f1w = f1s[l]
                xin = xf8_all[l] if FFN_FP8 else xn_all[l][0]
                c0, c1 = tiles[ti]
                Ct = c1 - c0
                if half == 0:
                    hh = sb.tile([128, FKC, 512], f8dt, tag="h", bufs=2)
                    hh_all[l][ti] = hh
                hh = hh_all[l][ti]
                for g in range(2 * half, 2 * half + 2):
                    ps = psp.tile([128, 4, 256], f32, tag="mm", bufs=3)
                    for j in range(KC):
                        mi = KC * g + j
                        first = (j % 2 == 0)
                        last = (j % 2 == 1)
                        for p in range(KC // 2):
                            nc.tensor.matmul(
                                ps[:, j, 0:Ct],
                                f1w[:, 2 * p:2 * p + 2,
                                    128 * mi:128 * (mi + 1)],
                                xin[:, 2 * p:2 * p + 2, c0:c1],
                                start=(p == 0 and first),
                                stop=(p == KC // 2 - 1 and last),
                                perf_mode=DR, skip_group_check=True)
                    if zb:
                        nc.scalar.activation(
                            hh[:, KC * g:KC * g + KC, 0:Ct], ps[:, :, 0:Ct],
                            AF.Gelu, scale=unsc)
                    else:
                        for j in range(KC):
                            mi = KC * g + j
                            nc.scalar.activation(
                                hh[:, mi, 0:Ct], ps[:, j, 0:Ct], AF.Gelu,
                                scale=unsc,
                                bias=par[:, pb + 4 + mi:pb + 4 + mi + 1])

            def f1a_stage(l, ti):
                _f1_half(l, ti, 0)

            def f1b_stage(l, ti):
                _f1_half(l, ti, 1)

            def f2_stage(l, ti):
                pb = _P_LAYER + 40 * l
                f2w = f2s[l]
                xn = xn_all[l][0]
                hh = hh_all[l][ti]
                c0, c1 = tiles[ti]
                Ct = c1 - c0
                y2 = sb.tile([128, KC, 512], bf16, tag="y", bufs=NT)
                ps = psp.tile([128, 4, 256], f32, tag="mm", bufs=3)
                for j in range(KC):
                    first = (j % 2 == 0)
                    last = (j % 2 == 1)
                    for p in range(FKC // 2):
                        nc.tensor.matmul(
                            ps[:, j, 0:Ct],
                            f2w[:, 2 * p:2 * p + 2, 128 * j:128 * (j + 1)],
                            hh[:, 2 * p:2 * p + 2, 0:Ct],
                            start=(p == 0 and first), stop=False,
                            perf_mode=DR, skip_group_check=True)
                    nc.tensor.matmul(
                        ps[:, j, 0:Ct], ident[:], xn[:, j, c0:c1],
                        start=False, stop=last, skip_group_check=True)
                if zb:
                    nc.scalar.activation(y2[:, :, 0:Ct], ps[:, :, 0:Ct],
                                         AF.Copy, scale=unsc)
                else:
                    for j in range(KC):
                        nc.scalar.activation(
                            y2[:, j, 0:Ct], ps[:, j, 0:Ct], AF.Copy,
                            scale=unsc,
                            bias=par[:, pb + 20 + j:pb + 20 + j + 1])
                y2s_all[l][ti] = y2

            def ln2_stage(l, ti):
                pb = _P_LAYER + 40 * l
                t = tiles[ti]
                st2 = stats_stage(y2s_all[l][ti], t[1] - t[0])
                ln_chain(y2s_all[l][ti], st2, t[1] - t[0], pb + 32, pb + 36,
                         xn_all[l][1],
                         xf8_all[l] if l == L - 1 else None, t,
                         newton=(l == L - 1))

            def out_stage(l, ti):
                xo = xf8_all[L - 1]
                c0, c1 = tiles[ti]
                Ct = c1 - c0
                ps = psp.tile([128, 4, 256], f32, tag="mm", bufs=3)
                for j in range(KC):
                    first = (j % 2 == 0)
                    last = (j % 2 == 1)
                    for p in range(KC // 2):
                        nc.tensor.matmul(
                            ps[:, j, 0:Ct],
                            opw[:, 2 * p:2 * p + 2, 128 * j:128 * (j + 1)],
                            xo[:, 2 * p:2 * p + 2, c0:c1],
                            start=(p == 0 and first),
                            stop=(p == KC // 2 - 1 and last),
                            perf_mode=DR, skip_group_check=True)
                ot = sb.tile([128, KC, 512], bf16, tag="o", bufs=2)
                if zb:
                    nc.scalar.activation(ot[:, :, 0:Ct], ps[:, :, 0:Ct],
                                         AF.Copy, scale=unsc)
                else:
                    for j in range(KC):
                        nc.scalar.activation(
                            ot[:, j, 0:Ct], ps[:, j, 0:Ct], AF.Copy,
                            scale=unsc,
                            bias=par[:, _P_OPB + j:_P_OPB + j + 1])
                for j in range(KC):
                    nc.sync.dma_start(
                        dram["out"].ap()[:, j * C + c0:j * C + c1],
                        ot[:, j, 0:Ct])

            # wavefront issue: stage ls of tile t at wave ls + t
            stage_fns = []
            for l in range(L):
                stage_fns += [
                    (attn_stage, l), (ln1_stage, l), (f1a_stage, l),
                    (f1b_stage, l), (f2_stage, l), (ln2_stage, l),
                ]
            stage_fns.append((out_stage, L - 1))
            NS = len(stage_fns)
            for wave in range(NS + NT - 1):
                for ls in range(NS - 1, -1, -1):
                    ti = wave - ls
                    if 0 <= ti < NT:
                        fn, l = stage_fns[ls]
                        fn(l, ti)

    nc.compile()
    return nc


_CACHE = {}


def _get_program(C, skips):
    key = (C, skips)
    if key not in _CACHE:
        _CACHE[key] = _build_program(C, skips)
    return _CACHE[key]


def _prep_gen_weights(i, g_ipw, g_ipb, g_qkv_w, g_qkv_b, g_ao_w, g_ao_b,
                      g_ln1g, g_ln1b, g_ln2g, g_ln2b, g_f1w, g_f1b, g_f2w,
                      g_f2b, g_opw, g_opb, g_rw):
    wa, ba = [], []
    for l in range(L):
        _wq, _wk, wv = np.split(g_qkv_w[i, l], 3, axis=0)
        _bq, _bk, bv = np.split(g_qkv_b[i, l], 3)
        wa.append((g_ao_w[i, l] @ wv).T)                 # [K=H, M=H]
        ba.append(g_ao_b[i, l] + bv @ g_ao_w[i, l].T)
    rw = float(g_rw[i])
    ws = W8SCALE if FFN_FP8 else 1.0
    f8np = mybir.dt.np(f8 if FFN_FP8 else bf16)
    ipP = _sb_pack(W8SCALE * g_ipw[i].T, mybir.dt.np(f8))
    opP = _sb_pack(W8SCALE * (1.0 - rw) * g_opw[i].T, mybir.dt.np(f8))
    waP = np.stack([_sb_pack(W8SCALE * wa[l], mybir.dt.np(f8))
                    for l in range(L)])
    f1P = np.stack([_sb_pack(ws * g_f1w[i, l].T, f8np) for l in range(L)])
    f2P = np.stack([_sb_pack(ws * g_f2w[i, l].T, f8np) for l in range(L)])

    par = np.zeros((128, 128), np.float32)
    par[:, _P_IPB:_P_IPB + KC] = _pack_pcol(g_ipb[i])
    for l in range(L):
        pb = _P_LAYER + 40 * l
        bal = ba[l] + (g_ipb[i] if l == 0 else 0.0)   # layer-0 fuses ipb
        par[:, pb:pb + 4] = _pack_pcol(bal)
        par[:, pb + 4:pb + 20] = _pack_pcol(g_f1b[i, l])
        par[:, pb + 20:pb + 24] = _pack_pcol(g_f2b[i, l])
        par[:, pb + 24:pb + 28] = _pack_pcol(g_ln1g[i, l])
        par[:, pb + 28:pb + 32] = _pack_pcol(g_ln1b[i, l])
        par[:, pb + 32:pb + 36] = _pack_pcol(g_ln2g[i, l])
        par[:, pb + 36:pb + 40] = _pack_pcol(g_ln2b[i, l])
    par[:, _P_OPB:_P_OPB + KC] = _pack_pcol((1.0 - rw) * g_opb[i])

    zb = bool(np.all(g_ipb[i] == 0) and all(np.all(b == 0) for b in ba)
              and np.all(g_f1b[i] == 0) and np.all(g_f2b[i] == 0)
              and np.all(g_opb[i] == 0))
    ug = bool(np.all(g_ln1g[i] == 1) and np.all(g_ln2g[i] == 1))
    zbeta = bool(np.all(g_ln1b[i] == 0) and np.all(g_ln2b[i] == 0))
    return {"ip": ipP, "op": opP, "wa": waP, "f1": f1P, "f2": f2P,
            "par": par}, (zb, ug, zbeta), rw


def _prepare(inputs):
    """Host-side prep. Returns (nc, in_maps, assemble)."""
    image = np.asarray(inputs["image_features"], np.float32)
    text = np.asarray(inputs["text_features"], np.float32)
    mt = np.asarray(inputs["missing_type"])

    idx1 = np.nonzero(mt == 1)[0]      # gen0 (img -> text) fills text
    idx2 = np.nonzero(mt == 2)[0]      # gen1 (text -> img) fills img
    idx3 = np.nonzero(mt == 3)[0]

    gw = {k: np.asarray(v) for k, v in inputs.items() if k.startswith("g_")}
    w0, skips0, rw0 = _prep_gen_weights(0, **gw)
    w1, skips1, rw1 = _prep_gen_weights(1, **gw)
    skips = tuple(a and b for a, b in zip(skips0, skips1))

    # prior MLP on host (tiny)
    pe = np.asarray(inputs["prior_emb"], np.float64)
    t = pe @ np.asarray(inputs["prior_w1"], np.float64).T \
        + np.asarray(inputs["prior_b1"], np.float64)
    t = 0.5 * t * (1.0 + np.vectorize(math.erf)(t / math.sqrt(2.0)))
    prior = (t @ np.asarray(inputs["prior_w2"], np.float64).T
             + np.asarray(inputs["prior_b2"], np.float64)).astype(np.float32)
    p_img, p_text = prior[0, :H], prior[0, H:]

    imgT = np.ascontiguousarray(image.T)
    textT = np.ascontiguousarray(text.T)

    n_pc = -(-max(len(idx1), len(idx2), 1) // GCORES)   # per-core columns
    C = max(64, -(-n_pc // 16) * 16)                    # round up to 16

    tls = _tiles(C)

    def _pack_tm(M):
        """[H, C] -> tile-major [128, NT*KC*Tt] fp8."""
        a = M.astype(mybir.dt.np(f8)).reshape(KC, 128, C).transpose(1, 0, 2)
        return np.concatenate(
            [np.ascontiguousarray(a[:, :, t0:t1]).reshape(128, -1)
             for t0, t1 in tls], axis=1)

    def shard_cols(Tsrc, Ttgt, idx):
        pad = np.zeros(GCORES * C, np.int64)
        pad[:len(idx)] = idx
        pad = pad.reshape(GCORES, C)
        return [_pack_tm(Tsrc[:, pad[c]]) for c in range(GCORES)], \
            [_pack_tm(Ttgt[:, pad[c]]) for c in range(GCORES)]

    src0, tgt0 = shard_cols(imgT, textT, idx1)
    src1, tgt1 = shard_cols(textT, imgT, idx2)

    nc = _get_program(C, skips)

    ones = np.ones((128, 128), ml_dtypes.bfloat16)
    ident = (np.eye(128, dtype=np.float32) * W8SCALE).astype(ml_dtypes.bfloat16)
    in_maps = []
    for c in range(N_CORES):
        g = 0 if c < GCORES else 1
        w = w0 if g == 0 else w1
        lc = c % GCORES
        in_maps.append({
            "src": (src0 if g == 0 else src1)[lc],
            "tgt": (tgt0 if g == 0 else tgt1)[lc],
            "ip": w["ip"], "op": w["op"], "wa": w["wa"], "f1": w["f1"],
            "f2": w["f2"],
            "par": w["par"], "ones": ones, "ident": ident,
        })

    def assemble(results):
        def gather_out(cores, idx, rw, full):
            cols = [np.asarray(results[c]["out"])
                    .astype(np.float32)
                    .reshape(128, KC, C).transpose(1, 0, 2).reshape(H, C)
                    for c in cores]
            allc = np.concatenate(cols, axis=1)[:, :len(idx)]
            return rw * full[idx] + allc.T

        enhanced_text = text.copy()
        if len(idx1):
            enhanced_text[idx1] = gather_out(range(GCORES), idx1, rw0, text)
        enhanced_img = image.copy()
        if len(idx2):
            enhanced_img[idx2] = gather_out(range(GCORES, N_CORES), idx2,
                                            rw1, image)
        if len(idx3):
            enhanced_img[idx3] = p_img
            enhanced_text[idx3] = p_text
        return enhanced_img, enhanced_text

    return nc, in_maps, assemble


def kernel(**inputs):
    nc, in_maps, assemble = _prepare(inputs)
    res = run_bass_kernel_spmd(nc, in_maps, list(range(N_CORES)))
    return assemble(res.results)
